# revision 1
# baseline (speedup 1.0000x reference)
"""Talking-heads attention Trainium2 kernel (8-core data-parallel over batch).

Reference computation (per batch item):
    q = x @ Wq ; k,v = x @ Wkv          (h=12 heads, d=64)
    S[h] = (q_h k_h^T) * d**-0.5
    S'[g] = sum_h mix_pre[h,g] S[h]     (talking-heads pre-softmax)
    P = softmax_j(S')
    P''[h] = sum_g mix_post[g,h] P[g]   (talking-heads post-softmax)
    out = concat_h(P''[h] @ v_h) @ Wo + bo

Design:
  * fp16 operands on the PE (fp32 PSUM accumulate); exp/softmax math fp32.
  * Logits are materialized partition-interleaved: 96 rows = 8 query rows x
    12 heads (each block padded to 128 for the DMA-transpose xbar, which
    folds output rows in fixed 128 blocks).  Both talking-heads mixes are
    then single 96x96 matmuls (block-structured mix matrices) per 512-wide
    j slab.
  * Softmax needs no max subtraction (logits are O(20)); exp runs on
    ScalarE with the fused per-partition sum (accum_out).  The 1/sum
    normalization is folded into the post-mix matrix (f32r, scaled per
    partition row by the DVE), costing one tiny [96,96] op per row group.
  * The partition interleave and the j-major transpose for attn@V use the
    DMA transpose crossbar (SBUF->SBUF, 2-byte), keeping VectorE/ScalarE
    free for the mandatory PSUM->SBUF evictions.
"""

import os
import numpy as np
import ml_dtypes

import concourse.bass as bass
import concourse.bacc as bacc
import concourse.mybir as mybir
import concourse.tile as tile
from concourse.bass_utils import run_bass_kernel_spmd
from contextlib import ExitStack

BF16_NP = ml_dtypes.bfloat16
FP16_NP = np.float16

F32 = mybir.dt.float32
BF16 = mybir.dt.bfloat16
FP16 = mybir.dt.float16
F32R = mybir.dt.float32r

# problem shape (hardcoded per contest rules)
B_TOTAL = 16
N_CORES = 8
B = B_TOTAL // N_CORES  # batch items per core
N = 1024                # sequence length
DIM = 768               # model dim
H = 12                  # heads
DH = 64                 # head dim
HC = H * DH             # 768
NK = DIM // 128         # 6 contraction tiles of 128
SCALE = DH ** -0.5

ILOC = 8                # query rows per row-group
GRP = ILOC * H          # 96 live partitions per row-group
CH = 128                # query rows per chunk
NIG = CH // ILOC        # 16 row-groups per chunk
NCH = N // CH           # 8 chunks per batch item
NJB = N // 128          # 8 key blocks
NCHP = N // 256         # 4 QK chunk pairs


def _copy(nc, idx, out, in_, scale=None):
    if idx % 2 == 0:
        if scale is None:
            nc.vector.tensor_copy(out, in_)
        else:
            nc.vector.tensor_scalar_mul(out, in_, scale)
    else:
        if scale is None:
            nc.scalar.copy(out, in_)
        else:
            nc.scalar.mul(out, in_, scale)


def build_program(debug_taps=(), reps=1):
    nc = bacc.Bacc(
        "TRN2",
        target_bir_lowering=False,
        debug=False,
        num_devices=N_CORES,
    )
    taps = {}
    for name, shape, dt in debug_taps:
        taps[name] = nc.declare_dram_parameter(name, list(shape), dt,
                                               isOutput=True)

    # all inputs pre-cast to fp16 host-side; bd matrices pre-built host-side
    x_d = nc.declare_dram_parameter("xb", [B, N, DIM], FP16, isOutput=False)
    wq_d = nc.declare_dram_parameter("wqb", [DIM, HC], FP16, isOutput=False)
    wk_d = nc.declare_dram_parameter("wkb", [DIM, HC], FP16, isOutput=False)
    wv_d = nc.declare_dram_parameter("wvb", [DIM, HC], FP16, isOutput=False)
    wo_d = nc.declare_dram_parameter("wob", [HC, DIM], FP16, isOutput=False)
    bdpre_d = nc.declare_dram_parameter("bdpre", [GRP, GRP], FP16, isOutput=False)
    bdpat_d = nc.declare_dram_parameter("bdpat", [GRP, GRP], F32, isOutput=False)
    bo_d = nc.declare_dram_parameter("bob", [1, DIM], FP16, isOutput=False)
    y_d = nc.declare_dram_parameter("y", [B, N, DIM], F32, isOutput=True)

    cc = [0]  # copy-engine alternation counter

    def nxt():
        cc[0] += 1
        return cc[0]

    with tile.TileContext(nc) as tc:
        with ExitStack() as ctx:
            persist = ctx.enter_context(tc.tile_pool(name="persist", bufs=1))
            sT_pool = ctx.enter_context(tc.tile_pool(name="sT", bufs=1))
            sInt_pool = ctx.enter_context(tc.tile_pool(name="sInt", bufs=1))
            e_pool = ctx.enter_context(tc.tile_pool(name="epool", bufs=2))
            p2_pool = ctx.enter_context(tc.tile_pool(name="p2pool", bufs=2))
            pt_pool = ctx.enter_context(tc.tile_pool(name="ptpool", bufs=1))
            small = ctx.enter_context(tc.tile_pool(name="small", bufs=4))
            y_pool = ctx.enter_context(tc.tile_pool(name="ypool", bufs=2))
            xbf_pool = ctx.enter_context(tc.tile_pool(name="xbf", bufs=1))

            ps_qkav = ctx.enter_context(
                tc.tile_pool(name="ps_qkav", bufs=2, space="PSUM"))
            ps_mix = ctx.enter_context(
                tc.tile_pool(name="ps_mix", bufs=2, space="PSUM"))
            ps_proj = ctx.enter_context(
                tc.tile_pool(name="ps_proj", bufs=1, space="PSUM"))
            ps_y = ctx.enter_context(
                tc.tile_pool(name="ps_y", bufs=1, space="PSUM"))

            # ---------------- persistent constants ----------------
            wo_t = persist.tile([128, NK, DIM], FP16, tag="wo")
            for kt in range(NK):
                nc.sync.dma_start(out=wo_t[:, kt, :],
                                  in_=wo_d[kt * 128:(kt + 1) * 128, :])
            bo_t = persist.tile([1, DIM], FP16, tag="bo")
            nc.sync.dma_start(out=bo_t[:], in_=bo_d[:, :])
            ones_t = persist.tile([1, CH], FP16, tag="ones")
            nc.vector.memset(ones_t[:], 1.0)
            bd_pre = persist.tile([GRP, GRP], FP16, tag="bdpre")
            nc.sync.dma_start(out=bd_pre[:], in_=bdpre_d[:, :])
            bd_pat = persist.tile([GRP, GRP], F32, tag="bdpat")
            nc.sync.dma_start(out=bd_pat[:], in_=bdpat_d[:, :])

            qT = persist.tile([128, NK, N], FP16, tag="qT")
            kT = persist.tile([128, NK, N], FP16, tag="kT")
            v_t = persist.tile([128, NJB, HC], FP16, tag="v")
            oaT = persist.tile([128, NK, N], FP16, tag="oaT")

            for rep in range(reps):
             for b in range(B):
                # qkv weights live in the sT slot; x^T lives in the sInt slot
                w3 = sT_pool.tile([128, 3, NK, HC], FP16, tag="sT")
                for kt in range(NK):
                    nc.sync.dma_start(out=w3[:, 0, kt, :],
                                      in_=wq_d[kt * 128:(kt + 1) * 128, :])
                    nc.sync.dma_start(out=w3[:, 1, kt, :],
                                      in_=wk_d[kt * 128:(kt + 1) * 128, :])
                    nc.sync.dma_start(out=w3[:, 2, kt, :],
                                      in_=wv_d[kt * 128:(kt + 1) * 128, :])
                xT = sInt_pool.tile([128, NK, N], FP16, tag="sInt")

                # ---------------- x load + transpose ----------------
                for ib in range(NCH):
                    x_bf = xbf_pool.tile([128, DIM], FP16, tag="xbf")
                    nc.sync.dma_start(
                        out=x_bf[:],
                        in_=x_d[b, ib * 128:(ib + 1) * 128, :])
                    nc.sync.dma_start_transpose(
                        out=xT[:, :, ib * 128:(ib + 1) * 128],
                        in_=x_bf[:])

                # ---------------- projections ----------------
                for wi, dst, scl in ((0, qT, SCALE), (1, kT, None)):
                    for ot in range(NK):        # output hc tile
                        for ic in range(2):     # i 512-slab
                            ps = ps_proj.tile([128, 512], F32, tag="proj")
                            for kt in range(NK):
                                nc.tensor.matmul(
                                    ps[:],
                                    lhsT=w3[:, wi, kt, ot * 128:(ot + 1) * 128],
                                    rhs=xT[:, kt, ic * 512:(ic + 1) * 512],
                                    start=(kt == 0), stop=(kt == NK - 1))
                            _copy(nc, nxt(), dst[:, ot, ic * 512:(ic + 1) * 512],
                                  ps[:], scale=scl)
                for jb in range(NJB):
                    for nh in range(2):         # v cols 0:512, 512:768
                        nw = 512 if nh == 0 else 256
                        ps = ps_proj.tile([128, 512], F32, tag="proj")
                        for kt in range(NK):
                            nc.tensor.matmul(
                                ps[:, 0:nw],
                                lhsT=xT[:, kt, jb * 128:(jb + 1) * 128],
                                rhs=w3[:, 2, kt, nh * 512:nh * 512 + nw],
                                start=(kt == 0), stop=(kt == NK - 1))
                        _copy(nc, nxt(), v_t[:, jb, nh * 512:nh * 512 + nw],
                              ps[:, 0:nw])

                if b == 0 and rep == 0:
                    for nm, src in (("dbg_qT", qT), ("dbg_kT", kT),
                                    ("dbg_v", v_t), ("dbg_xT", xT)):
                        if nm in taps:
                            nc.sync.dma_start(out=taps[nm][:], in_=src[:])

                # ---------------- attention ----------------
                for chp in range(NCHP):
                    # QK^T for a 256-row chunk pair, j on partitions.
                    # sT free layout: igrp blocks of 128 cols (col =
                    # (i%8)*12+h, 96:128 pad) -- xbar folds rows by 128.
                    sT = sT_pool.tile([128, NJB, 2 * NIG, 128], FP16, tag="sT")
                    mm_i = 0
                    ps = None
                    for h in range(H):
                        ht, hr = divmod(h, 2)
                        for jb in range(NJB):
                            slot = mm_i % 2
                            if slot == 0:
                                ps = ps_qkav.tile([128, 512], F32, tag="qkav")
                            nc.tensor.matmul(
                                ps[:, slot * 256:(slot + 1) * 256],
                                lhsT=kT[hr * 64:(hr + 1) * 64, ht,
                                        jb * 128:(jb + 1) * 128],
                                rhs=qT[hr * 64:(hr + 1) * 64, ht,
                                       chp * 256:(chp + 1) * 256],
                                start=True, stop=True)
                            mm_i += 1
                            if slot == 1:
                                # psum holds (h, jb-1) then (h, jb), each
                                # [128 j, 256 i] -> scatter into sT blocks
                                src = ps[:].rearrange(
                                    "p (s a c) -> p s a c", s=2, c=ILOC)
                                dst = sT[:, jb - 1:jb + 1, :, h:h + 96:12]
                                _copy(nc, nxt(), dst, src)

                    if b == 0 and chp == 0 and rep == 0 and "dbg_sT" in taps:
                        nc.sync.dma_start(out=taps["dbg_sT"][:], in_=sT[:])

                    for half in range(2):
                        ch = 2 * chp + half
                        # interleave: S_int[(i%8)*12+h, i//8, j] (96:128 junk)
                        sInt = sInt_pool.tile([128, NIG, N], FP16, tag="sInt")
                        for jb in range(NJB):
                            for _dup in range(1 + int(os.environ.get("KDUP1", "0"))):
                                nc.sync.dma_start_transpose(
                                    out=sInt[:, :, jb * 128:(jb + 1) * 128],
                                    in_=sT[:, jb, NIG * half:NIG * (half + 1), :])
                        if (b == 0 and ch == 0 and rep == 0
                                and "dbg_sInt" in taps):
                            nc.sync.dma_start(out=taps["dbg_sInt"][:],
                                              in_=sInt[:])

                        # per row-group: premix -> exp+sum -> postmix -> T
                        pT = pt_pool.tile([128, NJB, NIG, GRP], FP16, tag="pT")
                        for ig in range(NIG):
                            psm = ps_mix.tile([GRP, N], F32, tag="mix")
                            for jj in range(2):
                                nc.tensor.matmul(
                                    psm[:, jj * 512:(jj + 1) * 512],
                                    lhsT=bd_pre[:],
                                    rhs=sInt[0:GRP, ig,
                                             jj * 512:(jj + 1) * 512],
                                    start=True, stop=True)
                            e_sb = e_pool.tile([GRP, N], F32R, tag="esb")
                            ssum = small.tile([GRP, 1], F32, tag="ssum")
                            nc.scalar.activation(
                                e_sb[:], psm[:],
                                mybir.ActivationFunctionType.Exp,
                                accum_out=ssum[:])
                            recip = small.tile([GRP, 1], F32, tag="recip")
                            nc.vector.reciprocal(recip[:], ssum[:])
                            bd_ps = small.tile([GRP, GRP], F32R, tag="bdps")
                            nc.vector.tensor_scalar_mul(bd_ps[:], bd_pat[:],
                                                        recip[:])
                            if (b == 0 and ch == 0 and ig == 0 and rep == 0
                                    and "dbg_E" in taps):
                                nc.sync.dma_start(out=taps["dbg_E"][:],
                                                  in_=e_sb[:].bitcast(F32))
                                if "dbg_sums" in taps:
                                    nc.sync.dma_start(out=taps["dbg_sums"][:],
                                                      in_=ssum[:])
                            psp = ps_mix.tile([GRP, N], F32, tag="mix")
                            for jj in range(2):
                                nc.tensor.matmul(
                                    psp[:, jj * 512:(jj + 1) * 512],
                                    lhsT=bd_ps[:],
                                    rhs=e_sb[:, jj * 512:(jj + 1) * 512],
                                    start=True, stop=True)
                            p2 = p2_pool.tile([GRP, N], FP16, tag="p2")
                            _copy(nc, nxt(), p2[:], psp[:])
                            # transpose to P_T[j, (h*8+i_local)]
                            nc.sync.dma_start_transpose(
                                out=pT[:, :, ig, :], in_=p2[:])

                        if (b == 0 and ch == 0 and rep == 0
                                and "dbg_pT" in taps):
                            nc.sync.dma_start(out=taps["dbg_pT"][:], in_=pT[:])

                        # attn @ V (two heads share one psum, column tiling)
                        for hp in range(H // 2):
                            psa = ps_qkav.tile([128, 512], F32, tag="qkav")
                            for hh in range(2):
                                h = 2 * hp + hh
                                for jb in range(NJB):
                                    nc.tensor.matmul(
                                        psa[64 * hh:64 * (hh + 1), 0:128],
                                        lhsT=v_t[:, jb, h * 64:(h + 1) * 64],
                                        rhs=pT[:, jb, :, 8 * h:8 * h + 8],
                                        start=(jb == 0), stop=(jb == NJB - 1),
                                        tile_position=(0, 64 * hh))
                            _copy(nc, nxt(),
                                  oaT[:, hp, ch * 128:(ch + 1) * 128],
                                  psa[:, 0:128])

                        if (b == 0 and ch == NCH - 1 and rep == 0
                                and "dbg_oaT" in taps):
                            nc.sync.dma_start(out=taps["dbg_oaT"][:],
                                              in_=oaT[:])

                        # output projection + bias (two column halves)
                        y_sb = y_pool.tile([128, DIM], F32, tag="ysb")
                        for nh in range(2):
                            nw = 512 if nh == 0 else 256
                            psy = ps_y.tile([128, 512], F32, tag="yps")
                            for kt in range(NK):
                                nc.tensor.matmul(
                                    psy[:, 0:nw],
                                    lhsT=oaT[:, kt, ch * 128:(ch + 1) * 128],
                                    rhs=wo_t[:, kt, nh * 512:nh * 512 + nw],
                                    start=(kt == 0), stop=False)
                            nc.tensor.matmul(
                                psy[:, 0:nw], lhsT=ones_t[:],
                                rhs=bo_t[:, nh * 512:nh * 512 + nw],
                                start=False, stop=True)
                            _copy(nc, nxt(),
                                  y_sb[:, nh * 512:nh * 512 + nw],
                                  psy[:, 0:nw])
                        nc.sync.dma_start(
                            out=y_d[b, ch * 128:(ch + 1) * 128, :],
                            in_=y_sb[:])

    nc.compile()
    return nc


def host_prep(inputs):
    """Pre-cast weights to fp16 and build the block-structured mix matrices."""
    mix_pre = np.asarray(inputs["mix_pre"], dtype=np.float32)
    mix_post = np.asarray(inputs["mix_post"], dtype=np.float32)
    # bd_pre[(i*12+h), (i*12+g)] = mix_pre[h, g]
    bd_pre = np.zeros((GRP, GRP), dtype=np.float32)
    # bd_pat[(i*12+g), (h*8+i)] = mix_post[g, h]
    bd_pat = np.zeros((GRP, GRP), dtype=np.float32)
    for i in range(ILOC):
        bd_pre[12 * i:12 * i + 12, 12 * i:12 * i + 12] = mix_pre
        for h in range(H):
            bd_pat[12 * i:12 * i + 12, h * ILOC + i] = mix_post[:, h]
    wkv = np.asarray(inputs["Wkv"], dtype=np.float32)
    common = {
        "wqb": np.asarray(inputs["Wq"], dtype=np.float32).astype(FP16_NP),
        "wkb": np.ascontiguousarray(wkv[:, :HC]).astype(FP16_NP),
        "wvb": np.ascontiguousarray(wkv[:, HC:]).astype(FP16_NP),
        "wob": np.asarray(inputs["Wo"], dtype=np.float32).astype(FP16_NP),
        "bob": np.asarray(inputs["bo"], dtype=np.float32).reshape(1, DIM)
               .astype(FP16_NP),
        "bdpre": bd_pre.astype(FP16_NP),
        "bdpat": bd_pat,
    }
    return common


def kernel(**inputs):
    x = np.asarray(inputs["x"], dtype=np.float32).astype(FP16_NP)
    common = host_prep(inputs)
    nc = build_program()
    in_maps = []
    for c in range(N_CORES):
        m = dict(common)
        m["xb"] = np.ascontiguousarray(x[c * B:(c + 1) * B])
        in_maps.append(m)
    res = run_bass_kernel_spmd(nc, in_maps, list(range(N_CORES)))
    out = np.concatenate([res.results[c]["y"] for c in range(N_CORES)], axis=0)
    return out.astype(np.float32)


if __name__ == "__main__":
    rng = np.random.default_rng(0)
    ins = {
        "x": rng.standard_normal((B_TOTAL, N, DIM), dtype=np.float32),
        "Wq": rng.standard_normal((DIM, HC), dtype=np.float32) * DIM ** -0.5,
        "Wkv": rng.standard_normal((DIM, 2 * HC), dtype=np.float32) * DIM ** -0.5,
        "mix_pre": rng.standard_normal((H, H), dtype=np.float32),
        "mix_post": rng.standard_normal((H, H), dtype=np.float32),
        "Wo": rng.standard_normal((HC, DIM), dtype=np.float32) * HC ** -0.5,
        "bo": np.zeros(DIM, dtype=np.float32),
    }
    y = kernel(**ins)
    print("kernel output", y.shape, y.dtype, float(np.abs(y).max()))



# revision 35
# speedup vs baseline: 1102.6102x; 1102.6102x over previous
"""Talking-heads attention Trainium2 kernel (8-core data-parallel over batch).

Reference computation (per batch item):
    q = x @ Wq ; k,v = x @ Wkv          (h=12 heads, d=64)
    S[h] = (q_h k_h^T) * d**-0.5
    S'[g] = sum_h mix_pre[h,g] S[h]     (talking-heads pre-softmax)
    P = softmax_j(S')
    P''[h] = sum_g mix_post[g,h] P[g]   (talking-heads post-softmax)
    out = concat_h(P''[h] @ v_h) @ Wo + bo

Design:
  * fp16 operands on the PE (fp32 PSUM accumulate); exp/softmax math fp32.
  * Logits are materialized partition-interleaved, HEAD-major: 96 rows =
    12 heads x 8 query rows (row = h*8 + i%8), padded to 128 for the DMA
    transpose crossbar.  Both talking-heads mixes are then single 96x96
    matmuls per 512-wide j slab.
  * QK^T runs per key-block (jb) with one PSUM tile per head: adjacent
    heads execute on different PE row-groups CONCURRENTLY, so they must
    drain into different PSUM banks (same-bank pairing is a HW write
    collision).  Evictions write contiguous 8-col runs; the crossbar
    transpose to the interleaved layout runs per (jb, half) inside the
    QK loop for early overlap.
  * Softmax needs no max subtraction (logits are O(30), fp32 exp); exp on
    ScalarE with fused per-partition sum (accum_out); 1/sum is folded into
    the post-mix matrix (f32r, scaled per partition row by the DVE).
  * pT transposes are batched 2 row-groups per descriptor; PSUM is pooled
    (one shared 1-bank pool + a 2-bank premix pool) so row-group chains
    pipeline.
"""

import numpy as np
import ml_dtypes

import concourse.bass as bass
import concourse.bacc as bacc
import concourse.mybir as mybir
import concourse.tile as tile
from concourse.bass_utils import run_bass_kernel_spmd
from contextlib import ExitStack

BF16_NP = ml_dtypes.bfloat16
FP16_NP = np.float16

F32 = mybir.dt.float32
BF16 = mybir.dt.bfloat16
FP16 = mybir.dt.float16
F32R = mybir.dt.float32r

# problem shape (hardcoded per contest rules)
B_TOTAL = 16
N_CORES = 8
B = B_TOTAL // N_CORES  # batch items per core
N = 1024                # sequence length
DIM = 768               # model dim
H = 12                  # heads
DH = 64                 # head dim
HC = H * DH             # 768
NK = DIM // 128         # 6 contraction tiles of 128
SCALE = DH ** -0.5

ILOC = 8                # query rows per row-group
GRP = ILOC * H          # 96 live partitions per row-group
CH = 128                # query rows per chunk
NIG = CH // ILOC        # 16 row-groups per chunk
NCH = N // CH           # 8 chunks per batch item
NJB = N // 128          # 8 key blocks
NCHP = N // 256         # 4 QK chunk pairs

# QK PSUM packing groups: same-row-group heads share a PSUM tile; the two
# concurrent row-groups (tiles of a pair) use different tiles/banks.
QK_HEADS = [(0, 2, 4, 6), (1, 3, 5, 7), (8, 10), (9, 11)]
QK_COLBASE = [0, 32, 64, 80]
QK_SIGMA = {}
for _ti, _hs in enumerate(QK_HEADS):
    for _s, _hh in enumerate(_hs):
        QK_SIGMA[_hh] = QK_COLBASE[_ti] + _s * 8


def _copy(nc, idx, out, in_, scale=None):
    if idx % 2 == 0:
        if scale is None:
            nc.vector.tensor_copy(out, in_)
        else:
            nc.vector.tensor_scalar_mul(out, in_, scale)
    else:
        if scale is None:
            nc.scalar.copy(out, in_)
        else:
            nc.scalar.mul(out, in_, scale)


def build_program(reps=1, loop_reps=1):
    nc = bacc.Bacc(
        "TRN2",
        target_bir_lowering=False,
        debug=False,
        num_devices=N_CORES,
    )

    # all inputs pre-cast to fp16 host-side; bd matrices pre-built host-side
    x_d = nc.declare_dram_parameter("xb", [B, N, DIM], FP16, isOutput=False)
    wq_d = nc.declare_dram_parameter("wqb", [DIM, HC], FP16, isOutput=False)
    wk_d = nc.declare_dram_parameter("wkb", [DIM, HC], FP16, isOutput=False)
    wv_d = nc.declare_dram_parameter("wvb", [DIM, HC], FP16, isOutput=False)
    wo_d = nc.declare_dram_parameter("wob", [HC, DIM], FP16, isOutput=False)
    bdpre_d = nc.declare_dram_parameter("bdpre", [GRP, GRP], FP16, isOutput=False)
    bdpat_d = nc.declare_dram_parameter("bdpat", [GRP, GRP], F32, isOutput=False)
    bo_d = nc.declare_dram_parameter("bob", [1, DIM], FP16, isOutput=False)
    y_d = nc.declare_dram_parameter("y", [B, N, DIM], F32, isOutput=True)

    cc = [0]  # copy-engine alternation counter

    def nxt():
        cc[0] += 1
        return cc[0]

    with tile.TileContext(nc) as tc:
        with ExitStack() as ctx:
            persist = ctx.enter_context(tc.tile_pool(name="persist", bufs=1))
            big = ctx.enter_context(tc.tile_pool(name="big", bufs=2))
            st_pool = ctx.enter_context(tc.tile_pool(name="sT", bufs=3))
            sInt_pool = ctx.enter_context(tc.tile_pool(name="sInt", bufs=2))
            e_pool = ctx.enter_context(tc.tile_pool(name="epool", bufs=2))
            p2_pool = ctx.enter_context(tc.tile_pool(name="p2pool", bufs=2))
            small = ctx.enter_context(tc.tile_pool(name="small", bufs=4))
            y_pool = ctx.enter_context(tc.tile_pool(name="ypool", bufs=2))
            oa_pool = ctx.enter_context(tc.tile_pool(name="oapool", bufs=2))

            ps1 = ctx.enter_context(
                tc.tile_pool(name="ps1", bufs=4, space="PSUM"))
            psm_pool = ctx.enter_context(
                tc.tile_pool(name="psm", bufs=2, space="PSUM"))

            # ---------------- persistent constants ----------------
            wo_t = persist.tile([128, NK, DIM], FP16, tag="wo")
            for kt in range(NK):
                nc.sync.dma_start(out=wo_t[:, kt, :],
                                  in_=wo_d[kt * 128:(kt + 1) * 128, :])
            bo_t = persist.tile([1, DIM], FP16, tag="bo")
            nc.sync.dma_start(out=bo_t[:], in_=bo_d[:, :])
            ones_t = persist.tile([1, CH], FP16, tag="ones")
            nc.vector.memset(ones_t[:], 1.0)
            bd_pre = persist.tile([GRP, GRP], FP16, tag="bdpre")
            nc.sync.dma_start(out=bd_pre[:], in_=bdpre_d[:, :])
            bd_pat = persist.tile([GRP, GRP], F32, tag="bdpat")
            nc.sync.dma_start(out=bd_pat[:], in_=bdpat_d[:, :])

            qT = persist.tile([128, NK, N], FP16, tag="qT")
            kT = persist.tile([128, NK, N], FP16, tag="kT")
            v_t = persist.tile([128, NJB, HC], FP16, tag="v")

            if loop_reps > 1:
                ctx.enter_context(tc.For_i(0, loop_reps, 1))

            for rep in range(reps):
             for b in range(B):
                def emit_qk_jb(sInt_dst, ch_q, jb):
                    sTjb = st_pool.tile([128, NIG, 128], FP16, tag="sT",
                                        name="sTjb")
                    # pad cols 96:128 feed the xbar transpose; zero them on
                    # the otherwise-idle GpSimd engine
                    nc.gpsimd.memset(sTjb[:, :, GRP:128], 0.0)
                    for grp in ((0, 1), (2, 3)):
                        tiles = []
                        for k, ti in enumerate(grp):
                            heads = QK_HEADS[ti]
                            tiles.append(ps1.tile(
                                [128, 128 * len(heads)], F32, tag="ps",
                                name=f"qk{k}"))
                        for s in range(len(QK_HEADS[grp[0]])):
                            for k, ti in enumerate(grp):
                                h = QK_HEADS[ti][s]
                                ht, hr = divmod(h, 2)
                                nc.tensor.matmul(
                                    tiles[k][:, s * 128:(s + 1) * 128],
                                    lhsT=kT[hr * 64:(hr + 1) * 64, ht,
                                            jb * 128:(jb + 1) * 128],
                                    rhs=qT[hr * 64:(hr + 1) * 64, ht,
                                           ch_q * 128:(ch_q + 1) * 128],
                                    start=True, stop=True)
                        for k, ti in enumerate(grp):
                            ns = len(QK_HEADS[ti])
                            srcv = tiles[k][:].rearrange(
                                "p (s a c) -> p s a c", s=ns, c=ILOC)
                            dstv = sTjb[:, :,
                                        QK_COLBASE[ti]:
                                        QK_COLBASE[ti] + ns * 8].rearrange(
                                "p a (s c) -> p s a c", s=ns)
                            nc.vector.tensor_copy(dstv, srcv)
                    nc.sync.dma_start_transpose(
                        out=sInt_dst[:, :, jb * 128:(jb + 1) * 128],
                        in_=sTjb[:])

                # qkv weights share the "big" slot with pT (phase-disjoint)
                w3 = big.tile([128, 3, NK, HC], FP16, tag="big")
                for kt in range(NK):
                    nc.sync.dma_start(out=w3[:, 0, kt, :],
                                      in_=wq_d[kt * 128:(kt + 1) * 128, :])
                    nc.sync.dma_start(out=w3[:, 1, kt, :],
                                      in_=wk_d[kt * 128:(kt + 1) * 128, :])
                    nc.sync.dma_start(out=w3[:, 2, kt, :],
                                      in_=wv_d[kt * 128:(kt + 1) * 128, :])
                xT = sInt_pool.tile([128, NK, N], FP16, tag="sInt")

                # ---------------- x load + transpose ----------------
                for ib in range(NCH):
                    x_bf = oa_pool.tile([128, DIM], FP16, tag="oa")
                    nc.sync.dma_start(
                        out=x_bf[:],
                        in_=x_d[b, ib * 128:(ib + 1) * 128, :])
                    nc.sync.dma_start_transpose(
                        out=xT[:, :, ib * 128:(ib + 1) * 128],
                        in_=x_bf[:])

                # ---------------- projections ----------------
                for wi, dst, scl in ((0, qT, SCALE), (1, kT, None)):
                    for ot in range(NK):        # output hc tile
                        for ic in range(2):     # i 512-slab
                            ps = ps1.tile([128, 512], F32, tag="ps")
                            for kt in range(NK):
                                nc.tensor.matmul(
                                    ps[:],
                                    lhsT=w3[:, wi, kt, ot * 128:(ot + 1) * 128],
                                    rhs=xT[:, kt, ic * 512:(ic + 1) * 512],
                                    start=(kt == 0), stop=(kt == NK - 1))
                            if scl is None:
                                nc.vector.tensor_copy(
                                    dst[:, ot, ic * 512:(ic + 1) * 512], ps[:])
                            else:
                                nc.vector.tensor_scalar_mul(
                                    dst[:, ot, ic * 512:(ic + 1) * 512],
                                    ps[:], scl)
                sInt_cur = sInt_pool.tile([128, NIG, N], FP16, tag="sInt",
                                          name="sIntA")
                for jb in range(NJB):
                    for nh in range(2):         # v cols 0:512, 512:768
                        nw = 512 if nh == 0 else 256
                        ps = ps1.tile([128, 512], F32, tag="ps")
                        for kt in range(NK):
                            nc.tensor.matmul(
                                ps[:, 0:nw],
                                lhsT=xT[:, kt, jb * 128:(jb + 1) * 128],
                                rhs=w3[:, 2, kt, nh * 512:nh * 512 + nw],
                                start=(kt == 0), stop=(kt == NK - 1))
                        nc.vector.tensor_copy(
                            v_t[:, jb, nh * 512:nh * 512 + nw], ps[:, 0:nw])
                    # interleave chunk-0 QK into the v projection
                    emit_qk_jb(sInt_cur, 0, jb)

                # ---------------- attention ----------------
                # Software-pipelined per 128-query chunk: while the mix/
                # softmax chain of chunk ch runs, the QK^T + transpose of
                # chunk ch+1 is emitted one key-block per row-group pair,
                # keeping PE/ACT/DVE/DMA all busy.
                # QK PSUM packing: all SAME-row-group heads share a tile
                # (serial on PE); the two concurrent row-groups use
                # different tiles/banks (HW requirement).

                def emit_attnv_hp(pT_src, oa, hp):
                    psa = ps1.tile([128, 512], F32, tag="ps")
                    for hh in range(2):
                        h = 2 * hp + hh
                        for jb in range(NJB):
                            nc.tensor.matmul(
                                psa[64 * hh:64 * (hh + 1), 0:128],
                                lhsT=v_t[:, jb, h * 64:(h + 1) * 64],
                                rhs=pT_src[:, :, jb, 8 * h:8 * h + 8],
                                start=(jb == 0), stop=(jb == NJB - 1),
                                tile_position=(0, 64 * hh))
                    nc.vector.tensor_copy(oa[:, hp, :], psa[:, 0:128])

                def emit_y(oa, ch_y):
                    y_sb = y_pool.tile([128, DIM], F32, tag="ysb")
                    for nh in range(2):
                        nw = 512 if nh == 0 else 256
                        psy = ps1.tile([128, 512], F32, tag="ps")
                        for kt in range(NK):
                            nc.tensor.matmul(
                                psy[:, 0:nw],
                                lhsT=oa[:, kt, :],
                                rhs=wo_t[:, kt, nh * 512:nh * 512 + nw],
                                start=(kt == 0), stop=False)
                        nc.tensor.matmul(
                            psy[:, 0:nw], lhsT=ones_t[:],
                            rhs=bo_t[:, nh * 512:nh * 512 + nw],
                            start=False, stop=True)
                        nc.scalar.copy(y_sb[:, nh * 512:nh * 512 + nw],
                                       psy[:, 0:nw])
                    nc.sync.dma_start(
                        out=y_d[b, ch_y * 128:(ch_y + 1) * 128, :],
                        in_=y_sb[:])

                pT_prev = None
                for ch in range(NCH):
                    sInt = sInt_cur
                    if ch + 1 < NCH:
                        sInt_nxt = sInt_pool.tile([128, NIG, N], FP16,
                                                  tag="sInt",
                                                  name="sIntB")
                    # per row-group: premix -> exp+sum -> postmix -> T
                    pT = big.tile([128, NIG, NJB, GRP], FP16, tag="big")
                    if pT_prev is not None:
                        oa = oa_pool.tile([128, NK, 128], FP16, tag="oa")
                    for igp in range(NIG // 2):
                        p2 = p2_pool.tile([GRP, 2, N], FP16, tag="p2")
                        for s2 in range(2):
                            ig = 2 * igp + s2
                            psm = psm_pool.tile([GRP, N], F32, tag="mix")
                            for jj in range(2):
                                nc.tensor.matmul(
                                    psm[:, jj * 512:(jj + 1) * 512],
                                    lhsT=bd_pre[:],
                                    rhs=sInt[0:GRP, ig,
                                             jj * 512:(jj + 1) * 512],
                                    start=True, stop=True)
                            e_sb = e_pool.tile([GRP, N], F32R, tag="esb")
                            ssum = small.tile([GRP, 1], F32, tag="ssum")
                            nc.scalar.activation(
                                e_sb[:], psm[:],
                                mybir.ActivationFunctionType.Exp,
                                accum_out=ssum[:])
                            recip = small.tile([GRP, 1], F32, tag="recip")
                            nc.vector.reciprocal(recip[:], ssum[:])
                            bd_ps = small.tile([GRP, GRP], F32R, tag="bdps")
                            nc.vector.tensor_scalar_mul(bd_ps[:], bd_pat[:],
                                                        recip[:])
                            for jj in range(2):
                                psp = ps1.tile([128, 512], F32, tag="ps")
                                nc.tensor.matmul(
                                    psp[0:GRP, :],
                                    lhsT=bd_ps[:],
                                    rhs=e_sb[:, jj * 512:(jj + 1) * 512],
                                    start=True, stop=True)
                                _copy(nc, nxt(),
                                      p2[:, s2, jj * 512:(jj + 1) * 512],
                                      psp[0:GRP, :])
                        # transpose 2 row-groups at once: p2 [96, (s2, j)]
                        # -> pT[j%128, (s2, jb), row]; pT is ig-major so the
                        # 2-group region is contiguous in block order
                        nc.sync.dma_start_transpose(
                            out=pT[:, 2 * igp:2 * igp + 2, :, :],
                            in_=p2[:])
                        if ch + 1 < NCH:
                            emit_qk_jb(sInt_nxt, ch + 1, igp)
                        if pT_prev is not None and igp < H // 2:
                            emit_attnv_hp(pT_prev, oa, igp)

                    if pT_prev is not None:
                        emit_y(oa, ch - 1)
                    pT_prev = pT
                    if ch + 1 < NCH:
                        sInt_cur = sInt_nxt

                # drain: attnV + y for the last chunk
                oa = oa_pool.tile([128, NK, 128], FP16, tag="oa")
                for hp in range(H // 2):
                    emit_attnv_hp(pT_prev, oa, hp)
                emit_y(oa, NCH - 1)

    nc.compile()
    return nc


def host_prep(inputs):
    """Pre-cast weights to fp16 and build the block-structured mix matrices.

    Head-major interleave: partition/row p = h*8 + (i % 8).
      bd_pre[(h*8+i), (g*8+i)] = mix_pre[h, g]   (premix lhsT)
      bd_pat[(g*8+i), (h*8+i)] = mix_post[g, h]  (postmix lhsT pattern;
                                                  scaled by 1/rowsum at
                                                  runtime)
    """
    mix_pre = np.asarray(inputs["mix_pre"], dtype=np.float32)
    mix_post = np.asarray(inputs["mix_post"], dtype=np.float32)
    bd_pre = np.zeros((GRP, GRP), dtype=np.float32)
    bd_pat = np.zeros((GRP, GRP), dtype=np.float32)
    for i in range(ILOC):
        for h in range(H):
            for g in range(H):
                bd_pre[QK_SIGMA[h] + i, g * ILOC + i] = mix_pre[h, g]
                bd_pat[g * ILOC + i, h * ILOC + i] = mix_post[g, h]
    wkv = np.asarray(inputs["Wkv"], dtype=np.float32)
    common = {
        "wqb": np.asarray(inputs["Wq"], dtype=np.float32).astype(FP16_NP),
        "wkb": np.ascontiguousarray(wkv[:, :HC]).astype(FP16_NP),
        "wvb": np.ascontiguousarray(wkv[:, HC:]).astype(FP16_NP),
        "wob": np.asarray(inputs["Wo"], dtype=np.float32).astype(FP16_NP),
        "bob": np.asarray(inputs["bo"], dtype=np.float32).reshape(1, DIM)
               .astype(FP16_NP),
        "bdpre": bd_pre.astype(FP16_NP),
        "bdpat": bd_pat,
    }
    return common


def kernel(**inputs):
    x = np.asarray(inputs["x"], dtype=np.float32).astype(FP16_NP)
    common = host_prep(inputs)
    nc = build_program()
    in_maps = []
    for c in range(N_CORES):
        m = dict(common)
        m["xb"] = np.ascontiguousarray(x[c * B:(c + 1) * B])
        in_maps.append(m)
    res = run_bass_kernel_spmd(nc, in_maps, list(range(N_CORES)))
    out = np.concatenate([res.results[c]["y"] for c in range(N_CORES)], axis=0)
    return out.astype(np.float32)


if __name__ == "__main__":
    rng = np.random.default_rng(0)
    ins = {
        "x": rng.standard_normal((B_TOTAL, N, DIM), dtype=np.float32),
        "Wq": rng.standard_normal((DIM, HC), dtype=np.float32) * DIM ** -0.5,
        "Wkv": rng.standard_normal((DIM, 2 * HC), dtype=np.float32) * DIM ** -0.5,
        "mix_pre": rng.standard_normal((H, H), dtype=np.float32),
        "mix_post": rng.standard_normal((H, H), dtype=np.float32),
        "Wo": rng.standard_normal((HC, DIM), dtype=np.float32) * HC ** -0.5,
        "bo": np.zeros(DIM, dtype=np.float32),
    }
    y = kernel(**ins)
    print("kernel output", y.shape, y.dtype, float(np.abs(y).max()))
